# revision 15
# baseline (speedup 1.0000x reference)
"""Block self-attention (Gaussian kernel weights) Trainium2 Bass kernel, v2.

For each independent block of B=1024 rows of `features` [262144, 128]:
    w = exp(-(sq_i + sq_j - 2 x@x^T)/25.6);  out = (w @ x)/B
Blocks are data-parallel across 8 NeuronCores (32 blocks per core).

Key algebra: w = D_e A D_e with A = exp(2G/25.6) symmetric, e = exp(-sq/25.6).
  out_j = (e_j/B) * sum_i A_ij * (e_i x_i)
The diagonal i=j term equals x_j/B EXACTLY (exponents cancel in fp32), so no
diag masking and no separate x/B add is needed; only bf16 quantization of
A_jj/y_j (~0.1% rms) touches the dominant term.

exp work uses w-symmetry: A chunks (ci,cj) computed only for cj>=ci
(36 of 64 per block), packed into a trapezoid stream of 4608 fp32 in PSUM
(2x [128,1536] tiles rotating = 6 banks) -> 3 big ACT instrs per block.
Lower-triangle chunks are obtained by DMA-xbar transposes (SBUF->SBUF)
of the upper chunks into a [128, tgt, src, 128] mirror tile.

Per-core engine budget (32 blocks): PE ~171us (mm1-tri 15 MMs + mm2 22 MMs,
all bf16 N<=512), ACT ~161us (3 exp instrs + escale), DVE ~140us,
GPSIMD ~125us, DMA ~210us across 2 HW queues + SWDGE cast-load.
"""

import math
import os

os.environ.setdefault("NEURON_RT_RESET_CORES", "1")

import numpy as np

import concourse.bass as bass
import concourse.tile as tile
from concourse import bacc, mybir
from concourse.bass_utils import run_bass_kernel_spmd

N_TOTAL = 262144
D = 128
B = 1024
NCORES = 8
ROWS_PER_CORE = N_TOTAL // NCORES   # 32768
NB_FULL = ROWS_PER_CORE // B        # 32 blocks per core
C = B // 128                        # 8 row-chunks per block

F32 = mybir.dt.float32
BF16 = mybir.dt.bfloat16
FP16 = mybir.dt.float16

SIGMA2X2 = 2.0 * (D / 10.0)         # 25.6
G_SCALE = 2.0 / SIGMA2X2            # 0.078125
NEG_INV = -1.0 / SIGMA2X2           # -0.0390625
# outT is cast fp32->fp16 with a 1/OSC scale to keep away from fp16 max;
# escale carries the compensating OSC/B.
OSC = 16.0
LNB = math.log(OSC / B)             # escale = exp(-sq/25.6 + LNB) = e_j*OSC/B

EXP = mybir.ActivationFunctionType.Exp
MULT = mybir.AluOpType.mult

# trapezoid packing: row c covers cols [128c, 1024) => width 1024-128c
ROW_W = [B - 128 * c for c in range(C)]
CUM = [0]
for w in ROW_W:
    CUM.append(CUM[-1] + w)
PACK = CUM[C]                        # 4608
TILE_W = 1536
NT = PACK // TILE_W                  # 3 ACT tiles per block


def mm1_pieces():
    """(tile_idx, off_in_tile, row_c, xt_col_start, n) split at 512 banks."""
    ps = []
    for c in range(C):
        s = CUM[c]
        while s < CUM[c + 1]:
            e = min(CUM[c + 1], (s // 512 + 1) * 512)
            ps.append((s // TILE_W, s % TILE_W, c, 128 * c + (s - CUM[c]), e - s))
            s = e
    return ps


MM1_PIECES = mm1_pieces()            # 15 MMs
MM1_BY_TILE = [[p for p in MM1_PIECES if p[0] == t] for t in range(NT)]


def mm2_pieces():
    """Per i-chunk c: list of (kind, js, je) with kind 'mir'/'dir';
    plus per-bank start/stop flags computed over emission order."""
    per_c = []
    order = []
    for c in range(C):
        lst = []
        # mirror part: j in [0, 128c)
        if c >= 1:
            je_all = 128 * c
            s = 0
            while s < je_all:
                e = min(je_all, (s // 512 + 1) * 512)
                lst.append(("mir", s, e))
                s = e
        # direct part: j in [128c, 1024)
        s = 128 * c
        while s < B:
            e = min(B, (s // 512 + 1) * 512)
            lst.append(("dir", s, e))
            s = e
        per_c.append(lst)
        for i, p in enumerate(lst):
            order.append((c, i, p))
    # start/stop per output bank (0: j<512, 1: j>=512)
    flags = {}
    seen = {0: [], 1: []}
    for c, i, (kind, js, je) in order:
        bank = 0 if js < 512 else 1
        seen[bank].append((c, i))
    for bank in (0, 1):
        for pos, (c, i) in enumerate(seen[bank]):
            flags[(c, i)] = (pos == 0, pos == len(seen[bank]) - 1)
    return per_c, flags


MM2_PER_C, MM2_FLAGS = mm2_pieces()  # 22 MMs total

# which ACT tile finishes each row's packed segment (for mirror readiness)
ROW_END_TILE = [min((CUM[c + 1] - 1) // TILE_W, NT - 1) for c in range(C)]

# mirror queue assignment per source row (sync 'S' or scalar 'A');
# scalar-queue DMAs occupy the ScalarE stream, so only a small share goes there.
MIR_Q = {0: "S", 1: "S", 2: "S", 3: "S", 4: "S", 5: "S", 6: "S"}
TIN_Q = "S"   # in-transpose queue
N_TAIL_DVE = 6  # tail chunks on DVE (rest on GPSIMD)


def build(nb: int = NB_FULL) -> bacc.Bacc:
    rows = nb * B
    nc = bacc.Bacc("TRN2", target_bir_lowering=False, debug=False)

    fin = nc.dram_tensor("features", [rows, D], F32, kind="ExternalInput").ap()
    fout = nc.dram_tensor("out", [rows, D], F32, kind="ExternalOutput").ap()

    # row index = b*1024 + c*128 + p
    fin_v = fin.rearrange("(b c p) d -> b p c d", p=128, c=C)
    fout_v = fout.rearrange("(b c p) d -> b p c d", p=128, c=C)

    qmap = {}

    with tile.TileContext(nc) as tc:
        qmap["S"] = nc.sync
        qmap["A"] = nc.scalar
        with (
            tc.tile_pool(name="const", bufs=1) as cpool,
            tc.tile_pool(name="xr", bufs=3) as xrpool,
            tc.tile_pool(name="xt", bufs=2) as xtpool,
            tc.tile_pool(name="y", bufs=3) as ypool,
            tc.tile_pool(name="sq", bufs=4) as sqpool,
            tc.tile_pool(name="ap", bufs=2) as apool,    # packed A [128,4608] bf16
            tc.tile_pool(name="m2", bufs=2) as m2pool,   # mirror [128,8,8,128] bf16
            tc.tile_pool(name="ot", bufs=2) as otpool,   # outT_sb fp16
            tc.tile_pool(name="tr", bufs=2) as trpool,   # trd fp16
            tc.tile_pool(name="of", bufs=2) as ofpool,   # out_final fp32
            tc.tile_pool(name="gp", bufs=2, space="PSUM") as gpool,
            tc.tile_pool(name="acc", bufs=1, space="PSUM") as accpool,
        ):
            lnb = cpool.tile([128, 1], F32)
            nc.gpsimd.memset(lnb[:], LNB)

            state: dict[int, dict] = {}

            def load(b):
                xr = xrpool.tile([128, C, D], BF16)
                nc.gpsimd.dma_start(out=xr[:], in_=fin_v[b])  # SWDGE cast DMA
                state[b] = dict(xr=xr)

            def prep(b):
                st = state[b]
                xr = st["xr"]
                xsq = sqpool.tile([128, C * D], BF16, tag="xsq")
                nc.gpsimd.tensor_mul(
                    xsq[:], xr[:].rearrange("p c d -> p (c d)"),
                    xr[:].rearrange("p c d -> p (c d)"),
                )
                sqcol = sqpool.tile([128, C], F32, tag="sqc")
                nc.vector.tensor_reduce(
                    sqcol[:], xsq[:].rearrange("p (c d) -> p c d", d=D),
                    axis=mybir.AxisListType.X, op=mybir.AluOpType.add,
                )
                bias_col = sqpool.tile([128, C], F32, tag="bia")
                nc.vector.tensor_scalar_mul(bias_col[:], sqcol[:], NEG_INV)
                st["bias_col"] = bias_col

            def escalc(b):
                st = state[b]
                escale = sqpool.tile([128, C], F32, tag="esc")
                nc.scalar.activation(escale[:], st.pop("bias_col")[:], EXP, bias=lnb[:])
                st["escale"] = escale

            def ymul(b):
                st = state[b]
                y = ypool.tile([128, C, D], BF16, tag="y")
                for c in range(C):
                    # y = xr * e_i  (escale*B/OSC = e_i)
                    nc.vector.tensor_scalar(
                        out=y[:, c, :], in0=st["xr"][:, c, :],
                        scalar1=st["escale"][:, c:c + 1], scalar2=float(B / OSC),
                        op0=MULT, op1=MULT,
                    )
                st["y"] = y

            def tin(b):
                st = state[b]
                xT = xtpool.tile([128, C, 128], BF16)
                qmap[TIN_Q].dma_start_transpose(
                    out=xT[:], in_=st["xr"][:].rearrange("p c d -> p (c d)")
                )
                st["xT"] = xT

            def m1_tile(b, t):
                st = state[b]
                if t == 0:
                    st["g"] = {}
                    st["apk"] = apool.tile([128, PACK], BF16, name="apk", tag="apk")
                g = gpool.tile([128, TILE_W], F32, tag="g")
                st["g"][t] = g
                xT = st["xT"][:].rearrange("p c d -> p (c d)")
                for (_, off, c, col, n) in MM1_BY_TILE[t]:
                    nc.tensor.matmul(
                        g[:, off:off + n],
                        lhsT=st["xT"][:, c, :],
                        rhs=xT[:, col:col + n],
                        start=True, stop=True,
                    )

            def act_tile(b, t):
                st = state[b]
                g = st["g"].pop(t)
                nc.scalar.activation(
                    st["apk"][:, t * TILE_W:(t + 1) * TILE_W], g[:], EXP,
                    scale=G_SCALE,
                )

            def mir(b, ci):
                st = state[b]
                if ci == 0:
                    st["m2"] = m2pool.tile([128, C, C, 128], BF16, name="m2", tag="m2")
                qmap[MIR_Q[ci]].dma_start_transpose(
                    out=st["m2"][:, ci + 1:C, ci, :],
                    in_=st["apk"][:, CUM[ci] + 128:CUM[ci + 1]],
                )

            def mm2_grp(b, cs):
                st = state[b]
                if 0 in cs:
                    o0 = accpool.tile([128, 512], F32, tag="o0")
                    o1 = accpool.tile([128, 512], F32, tag="o1")
                    st["o"] = [o0, o1]
                for c in cs:
                    for i, (kind, js, je) in enumerate(MM2_PER_C[c]):
                        start, stop = MM2_FLAGS[(c, i)]
                        bank = 0 if js < 512 else 1
                        ob = st["o"][bank]
                        if kind == "mir":
                            rhs = st["m2"][:, c, js // 128:je // 128, :]
                            rhs = rhs.rearrange("p s d -> p (s d)")
                        else:
                            lo = CUM[c] + (js - 128 * c)
                            rhs = st["apk"][:, lo:lo + (je - js)]
                        nc.tensor.matmul(
                            ob[:, js - 512 * bank:je - 512 * bank],
                            lhsT=st["y"][:, c, :],
                            rhs=rhs,
                            start=start, stop=stop,
                        )

            def casts(b):
                st = state[b]
                ot = otpool.tile([128, B], FP16)
                for h in range(2):
                    nc.vector.tensor_scalar_mul(
                        ot[:, h * 512:(h + 1) * 512], st["o"][h][:], 1.0 / OSC
                    )
                st["ot"] = ot

            def tout(b):
                st = state[b]
                trd = trpool.tile([128, C, 128], FP16)
                nc.sync.dma_start_transpose(out=trd[:], in_=st["ot"][:])
                st["trd"] = trd

            def tail(b):
                st = state[b]
                of = ofpool.tile([128, C, D], F32)
                for c in range(C):
                    nc.vector.tensor_scalar_mul(
                        of[:, c, :], st["trd"][:, c, :], st["escale"][:, c:c + 1]
                    )
                st["of"] = of

            def store(b):
                st = state.pop(b)
                nc.sync.dma_start(out=fout_v[b], in_=st["of"][:])

            # software pipeline: iteration k handles load(k), prep(k-1),
            # m1/act/mir(k-2), mm2/epilogue(k-3).  ACT-feeding work first;
            # escale/T_in at the end so they never block the exp stream.
            for k in range(nb + 3):
                bl, bp, bm, be = k, k - 1, k - 2, k - 3
                if bl < nb:
                    load(bl)
                if 0 <= bp < nb:
                    prep(bp)
                    tin(bp)
                if 0 <= bm < nb:
                    m1_tile(bm, 0)
                    act_tile(bm, 0)
                if 0 <= be < nb:
                    mm2_grp(be, [0, 1, 2, 3])
                if 0 <= bm < nb:
                    mir(bm, 0)
                    m1_tile(bm, 1)
                    act_tile(bm, 1)
                    mir(bm, 1)
                    mir(bm, 2)
                if 0 <= be < nb:
                    mm2_grp(be, [4, 5])
                if 0 <= bm < nb:
                    m1_tile(bm, 2)
                    act_tile(bm, 2)
                if 0 <= be < nb:
                    mm2_grp(be, [6, 7])
                    casts(be)
                    tout(be)
                if 0 <= bm < nb:
                    for ci in range(3, 7):
                        mir(bm, ci)
                if 0 <= bp < nb:
                    escalc(bp)
                    ymul(bp)
                if 0 <= be < nb:
                    tail(be)
                    store(be)

    nc.compile()
    return nc


_CACHE: dict[int, bacc.Bacc] = {}


def _get_nc(nb: int = NB_FULL) -> bacc.Bacc:
    if nb not in _CACHE:
        _CACHE[nb] = build(nb)
    return _CACHE[nb]


def run(features: np.ndarray, nc: bacc.Bacc | None = None, **spmd_kwargs):
    """Shard rows across 8 cores, run, gather. Returns (out, BassKernelResults)."""
    features = np.ascontiguousarray(features, dtype=np.float32)
    assert features.shape == (N_TOTAL, D)
    if nc is None:
        nc = _get_nc()
    core_ids = list(range(NCORES))
    shards = np.split(features, NCORES, axis=0)
    in_maps = [{"features": s} for s in shards]
    res = run_bass_kernel_spmd(nc, in_maps, core_ids, **spmd_kwargs)
    out = np.concatenate([res.results[i]["out"] for i in range(NCORES)], axis=0)
    return out, res


def kernel(features: np.ndarray) -> np.ndarray:
    out, _ = run(features)
    return out
